# revision 1
# baseline (speedup 1.0000x reference)
"""PillarVFE on 8 trn2 NeuronCores — v3: fp16 matmuls + raw chain-max,
epilogue on host.

Math: per pillar p, point n with raw r=(x,y,z,w):
  out[p,o] = relu( max( max_n (r_n . A)[o] - Q_p[o],  C_p[o] ) )
where A[4,64] folds W + BN scale, Q_p folds the pillar-constant part
(center offsets + cluster mean) minus the BN bias, and C_p is the
candidate from masked points: c0 if npts<32 else -inf.  The device
computes only S_p[o] = max_n (r_n . A)[o]; the cheap elementwise
epilogue (pair fold, -Q, max C, relu, unpermute) runs on host.
Invalid points' raw data is replaced host-side by point 0 (always
valid), so their scores never change the max.

Sharding: pillars sorted by npts descending, padded to 40960, dealt as
80 chunks of 512 round-robin over 8 cores.  Slot i (chunk 8i+k on core
k) computes ceil(maxN_i/2) point-pair matmuls, where maxN_i = npts of
the first pillar of chunk 8i — a shared slot schedule, so one SPMD
program serves all cores.  Per pair: one K=32 fp16 matmul (stationary
selects 2 points -> M=128: even point -> partitions 0..63, odd ->
64..127); DVE folds each PSUM bank into the running SBUF max (first
bank: copy).  Output per slot: raw [128,512] max, DMA'd out.
"""

import sys

import numpy as np

sys.path.insert(0, "/opt/trn_rl_repo")

VX, VY = 0.16, 0.16
X_OFF = VX / 2 + 0.0
Y_OFF = VY / 2 + (-39.68)
BN_EPS = 1e-3

P_FULL = 40000
N_PTS = 32
C_OUT = 64
N_CORES = 8
N_SLOTS = 10
TILE_P = 512
P_PAD = N_CORES * N_SLOTS * TILE_P  # 40960

_CACHE = {}


def _build_nc(sched):
    from contextlib import ExitStack

    from concourse import bass, tile
    from concourse import mybir

    f32 = mybir.dt.float32
    f16 = mybir.dt.float16
    nc = bass.Bass()

    T_ds = []
    for i, maxN in enumerate(sched):
        G = (maxN + 7) // 8
        T_ds.append(
            nc.dram_tensor(f"T{i}", [32 * G, TILE_P], f16, kind="ExternalInput")
        )
    S_d = nc.dram_tensor("S", [128, 4, 128], f16, kind="ExternalInput")
    O_d = nc.dram_tensor("O", [N_SLOTS, 128, TILE_P], f32, kind="ExternalOutput")

    with tile.TileContext(nc) as tc, ExitStack() as ctx:
        stat = ctx.enter_context(tc.tile_pool(name="stat", bufs=1))
        tpool = ctx.enter_context(tc.tile_pool(name="tin", bufs=2))
        work = ctx.enter_context(tc.tile_pool(name="work", bufs=3))
        psum = ctx.enter_context(
            tc.tile_pool(name="ps", bufs=8, space=bass.MemorySpace.PSUM)
        )

        s_sb = stat.tile([128, 4, 128], f16)
        nc.sync.dma_start(s_sb[:], S_d[:])

        for i, maxN in enumerate(sched):
            G = (maxN + 7) // 8
            n = (maxN + 1) // 2
            t_sb = tpool.tile([32 * G, TILE_P], f16)
            nc.sync.dma_start(t_sb[:], T_ds[i][:])

            pairs = [
                (w, g) for w in range(4) for g in range(G) if 8 * g + 2 * w < maxN
            ]
            assert len(pairs) == n, (i, maxN, pairs)
            prev = None
            for w, g in pairs:
                b = psum.tile([128, TILE_P], f32)
                nc.tensor.matmul(
                    b[:],
                    s_sb[32 * g : 32 * g + 32, w, :],
                    t_sb[32 * g : 32 * g + 32, :],
                    start=True,
                    stop=True,
                    tile_position=(32 * g, 0),
                )
                cur = work.tile([128, TILE_P], f32)
                if prev is None:
                    nc.vector.tensor_copy(cur[:], b[:])
                else:
                    nc.vector.tensor_max(cur[:], prev[:], b[:])
                prev = cur
            nc.sync.dma_start(O_d[i], prev[:])

    nc.finalize()
    import bass_rust

    # walrus codegen allows at most 1 sync wait per instruction
    bass_rust.generate_event_semaphores(nc)
    return nc


def _plan(voxels, W, gamma, beta, running_mean, running_var,
          voxel_num_points, voxel_coords):
    V = voxels.astype(np.float64)
    npts = voxel_num_points.astype(np.int64)
    coords = voxel_coords.astype(np.float64)
    W64 = W.astype(np.float64)
    s = gamma.astype(np.float64) / np.sqrt(running_var.astype(np.float64) + BN_EPS)
    c0 = beta.astype(np.float64) - running_mean.astype(np.float64) * s

    A = np.stack([
        s * (W64[:, 0] + W64[:, 4] + W64[:, 7]),
        s * (W64[:, 1] + W64[:, 5] + W64[:, 8]),
        s * (W64[:, 2] + W64[:, 6]),
        s * W64[:, 3],
    ], axis=0)  # [4,64]

    cx = coords[:, 3] * VX + X_OFF
    cy = coords[:, 2] * VY + Y_OFF
    m = V[:, :, :3].sum(axis=1) / npts[:, None]
    q = (cx[:, None] * (s * (W64[:, 0] + W64[:, 7]))[None, :]
         + cy[:, None] * (s * (W64[:, 1] + W64[:, 8]))[None, :]
         + m[:, 0:1] * (s * W64[:, 4])[None, :]
         + m[:, 1:2] * (s * W64[:, 5])[None, :]
         + m[:, 2:3] * (s * W64[:, 6])[None, :])
    Q = (q - c0[None, :]).astype(np.float32)                    # [P,64]
    C = np.where((npts < N_PTS)[:, None], c0[None, :], -1e30).astype(np.float32)

    Vmod = voxels.astype(np.float16).copy()
    invalid = np.arange(N_PTS)[None, :] >= npts[:, None]
    Vmod[invalid] = np.broadcast_to(Vmod[:, 0:1, :], Vmod.shape)[invalid]

    pad = P_PAD - P_FULL
    Vp = np.concatenate([Vmod, np.zeros((pad, N_PTS, 4), np.float16)], axis=0)
    Qp = np.concatenate([Q, np.zeros((pad, C_OUT), np.float32)], axis=0)
    Cp = np.concatenate([C, np.zeros((pad, C_OUT), np.float32)], axis=0)
    np_pad = np.concatenate([npts, np.ones(pad, np.int64)])

    order = np.argsort(-np_pad, kind="stable")
    ns = np_pad[order]
    sched = tuple(int(ns[N_CORES * TILE_P * i]) for i in range(N_SLOTS))

    # stationaries: S[32g+4j+c, w, m] = A[c, m%64] if j == 2w + m//64
    A16 = A.astype(np.float16)
    S_small = np.zeros((32, 4, 128), np.float16)
    for w in range(4):
        for half in range(2):
            j = 2 * w + half
            S_small[4 * j : 4 * j + 4, w, 64 * half : 64 * half + 64] = A16
    S = np.tile(S_small, (4, 1, 1))  # [128,4,128]

    Vs = Vp[order]
    in_maps = []
    for k in range(N_CORES):
        mp = {"S": S}
        for i, maxN in enumerate(sched):
            G = (maxN + 7) // 8
            c = N_CORES * i + k
            sl = slice(TILE_P * c, TILE_P * (c + 1))
            mp[f"T{i}"] = np.ascontiguousarray(
                Vs[sl][:, : 8 * G, :].transpose(1, 2, 0).reshape(32 * G, TILE_P)
            )
        in_maps.append(mp)
    return in_maps, sched, order, Qp[order], Cp[order]


def _gather(results, order, Qs, Cs):
    smax = np.empty((P_PAD, C_OUT), np.float32)
    for k in range(N_CORES):
        Ok = results[k]["O"]  # [10,128,512]
        for i in range(N_SLOTS):
            c = N_CORES * i + k
            fold = np.maximum(Ok[i, :C_OUT, :], Ok[i, C_OUT:, :])
            smax[TILE_P * c : TILE_P * (c + 1)] = fold.T
    out_sorted = np.maximum(np.maximum(smax - Qs, Cs), 0.0)
    out_full = np.empty_like(out_sorted)
    out_full[order] = out_sorted
    return np.ascontiguousarray(out_full[:P_FULL])


def kernel(**inputs):
    from concourse.bass_utils import run_bass_kernel_spmd

    in_maps, sched, order, Qs, Cs = _plan(**inputs)
    if sched not in _CACHE:
        _CACHE[sched] = _build_nc(sched)
    res = run_bass_kernel_spmd(_CACHE[sched], in_maps, list(range(N_CORES)))
    return _gather(res.results, order, Qs, Cs)



# revision 8
# speedup vs baseline: 1.8445x; 1.8445x over previous
"""PillarVFE on 8 trn2 NeuronCores — v5: fp16 matmuls + dual-engine PSUM
drain (DVE reduce / Act cast + DVE fp16 fold), plane outputs, epilogue on
host.

Math: per pillar p, point n with raw r=(x,y,z,w):
  out[p,o] = relu( max( max_n (r_n . A)[o] - Q_p[o],  C_p[o] ) )
where A[4,64] folds W + BN scale, Q_p folds the pillar-constant part
(center offsets + cluster mean) minus the BN bias, and C_p is the
candidate from masked points: c0 if npts<32 else -inf.  The device
computes partial maxes of S_p[o] = max_n (r_n . A)[o]; the cheap
elementwise epilogue (plane fold, half fold, -Q, max C, relu,
unpermute) runs on host.

Device structure per slot (B = ceil(maxN/2) point-pair matmuls):
matmuls fill [128,<=4,512] PSUM tiles (2-pt-per-column trick: partition
= 2x64 channels, free = 512 pillars).  Each PSUM tile is drained by ONE
of: DVE tensor_reduce(max) over a transposed view -> one fp16 plane, or
Act copy-cast -> contiguous fp16 tile which DVE halving-folds at 2x ->
1-2 planes.  Hardware constraints found empirically: TensorTensor reads
at most one PSUM operand, GPSIMD cannot touch PSUM or run TensorTensor,
cross-partition folds are impossible (lane-locked), strided Act writes
are 5x slow, fp16 TensorTensor gets 2x mode, reduce does not.
Planes accumulate in a per-slot out tile, DMA'd to HBM; host folds.
"""

import sys

import numpy as np

sys.path.insert(0, "/opt/trn_rl_repo")

VX, VY = 0.16, 0.16
X_OFF = VX / 2 + 0.0
Y_OFF = VY / 2 + (-39.68)
BN_EPS = 1e-3

P_FULL = 40000
N_PTS = 32
C_OUT = 64
N_CORES = 8
N_SLOTS = 10
TILE_P = 512
P_PAD = N_CORES * N_SLOTS * TILE_P  # 40960

_CACHE = {}


# measured op costs (ns) for the engine load balancer
def _cv_reduce(b):   # DVE psum reduce over [128,512,b]
    return 533.0 * b + 157.0


def _ca_cast(b):     # Act contiguous cast [128, b*512]
    return 427.0 * b + 258.0


def _cv_fold(b):     # DVE fp16 fold tax for an Act tile of b banks
    return {1: 0.0, 2: 390.0, 3: 684.0, 4: 684.0}[b]


def _slot_tiles(maxN):
    B = (maxN + 1) // 2
    out = []
    while B > 0:
        b = min(4, B)
        out.append(b)
        B -= b
    return out


def _plan_slots(sched):
    """Greedy DVE/Act assignment per psum tile; returns per-slot
    (n_planes, owners)."""
    est_v, est_a = 0.0, 0.0
    plan = []
    for maxN in sched:
        owners = []
        n_planes = 0
        for b in _slot_tiles(maxN):
            cost_v = _cv_reduce(b)
            cost_a = _ca_cast(b)
            tax_v = _cv_fold(b)
            mk_v = max(est_v + cost_v, est_a)
            mk_a = max(est_v + tax_v, est_a + cost_a)
            if mk_v <= mk_a:
                owners.append("v")
                est_v += cost_v
                n_planes += 1
            else:
                owners.append("a")
                est_a += cost_a
                est_v += tax_v
                n_planes += 1 if b <= 2 else 2
        plan.append((n_planes, owners))
    _plan_slots.est = (est_v, est_a)
    return plan


def _build_nc(sched):
    from contextlib import ExitStack

    from concourse import bass, tile
    from concourse import mybir

    f32 = mybir.dt.float32
    f16 = mybir.dt.float16
    nc = bass.Bass()

    plan = _plan_slots(sched)

    T_ds = []
    for i, maxN in enumerate(sched):
        G = (maxN + 7) // 8
        T_ds.append(
            nc.dram_tensor(f"T{i}", [32 * G, TILE_P], f16, kind="ExternalInput")
        )
    S_d = nc.dram_tensor("S", [128, 4, 128], f16, kind="ExternalInput")
    O_ds = [
        nc.dram_tensor(f"O{i}", [128, n_planes, TILE_P], f16, kind="ExternalOutput")
        for i, (n_planes, _) in enumerate(plan)
    ]

    with tile.TileContext(nc) as tc, ExitStack() as ctx:
        stat = ctx.enter_context(tc.tile_pool(name="stat", bufs=1))
        tpool = ctx.enter_context(tc.tile_pool(name="tin", bufs=2))
        fold = ctx.enter_context(tc.tile_pool(name="fold", bufs=4))
        opool = ctx.enter_context(tc.tile_pool(name="opool", bufs=2))
        psum = ctx.enter_context(
            tc.tile_pool(name="ps", bufs=2, space=bass.MemorySpace.PSUM)
        )

        s_sb = stat.tile([128, 4, 128], f16)
        nc.sync.dma_start(s_sb[:], S_d[:])

        for i, maxN in enumerate(sched):
            G = (maxN + 7) // 8
            B = (maxN + 1) // 2
            n_planes, owners = plan[i]
            t_sb = tpool.tile([32 * G, TILE_P], f16, name="t")
            nc.sync.dma_start(t_sb[:], T_ds[i][:])

            pairs = [
                (w, g) for w in range(4) for g in range(G) if 8 * g + 2 * w < maxN
            ]
            assert len(pairs) == B, (i, maxN, pairs)

            out_sb = opool.tile([128, n_planes, TILE_P], f16, name="o")
            idx = 0
            j = 0
            for ti, b in enumerate(_slot_tiles(maxN)):
                pt = psum.tile([128, 4, TILE_P], f32, name="pt")
                for jj in range(b):
                    w, g = pairs[j]
                    nc.tensor.matmul(
                        pt[:, jj, :],
                        s_sb[32 * g : 32 * g + 32, w, :],
                        t_sb[32 * g : 32 * g + 32, :],
                        start=True,
                        stop=True,
                        tile_position=(32 * g, 0),
                    )
                    j += 1
                if owners[ti] == "v":
                    if b == 1:
                        nc.vector.tensor_copy(out_sb[:, idx, :], pt[:, 0, :])
                    else:
                        nc.vector.tensor_reduce(
                            out_sb[:, idx, :],
                            pt[:, 0:b, :].transpose([0, 2, 1]),
                            axis=mybir.AxisListType.X,
                            op=mybir.AluOpType.max,
                        )
                    idx += 1
                else:
                    if b == 1:
                        nc.scalar.copy(out_sb[:, idx, :], pt[:, 0, :])
                        idx += 1
                    else:
                        u = fold.tile([128, 4, TILE_P], f16, name="u")
                        nc.scalar.copy(u[:, 0:b, :], pt[:, 0:b, :])
                        if b == 2:
                            nc.vector.tensor_max(
                                out_sb[:, idx, :], u[:, 0, :], u[:, 1, :]
                            )
                            idx += 1
                        else:  # b in (3, 4): fold to 2 planes (overlap for 3)
                            lo = b - 2
                            nc.vector.tensor_max(
                                out_sb[:, idx : idx + 2, :],
                                u[:, 0:2, :],
                                u[:, lo : lo + 2, :],
                            )
                            idx += 2
            assert idx == n_planes, (i, idx, n_planes)
            nc.sync.dma_start(O_ds[i][:], out_sb[:])

    nc.finalize()
    import bass_rust

    # walrus codegen allows at most 1 sync wait per instruction
    bass_rust.generate_event_semaphores(nc)
    return nc


def _plan(voxels, W, gamma, beta, running_mean, running_var,
          voxel_num_points, voxel_coords):
    V = voxels.astype(np.float64)
    npts = voxel_num_points.astype(np.int64)
    coords = voxel_coords.astype(np.float64)
    W64 = W.astype(np.float64)
    s = gamma.astype(np.float64) / np.sqrt(running_var.astype(np.float64) + BN_EPS)
    c0 = beta.astype(np.float64) - running_mean.astype(np.float64) * s

    A = np.stack([
        s * (W64[:, 0] + W64[:, 4] + W64[:, 7]),
        s * (W64[:, 1] + W64[:, 5] + W64[:, 8]),
        s * (W64[:, 2] + W64[:, 6]),
        s * W64[:, 3],
    ], axis=0)  # [4,64]

    cx = coords[:, 3] * VX + X_OFF
    cy = coords[:, 2] * VY + Y_OFF
    m = V[:, :, :3].sum(axis=1) / npts[:, None]
    q = (cx[:, None] * (s * (W64[:, 0] + W64[:, 7]))[None, :]
         + cy[:, None] * (s * (W64[:, 1] + W64[:, 8]))[None, :]
         + m[:, 0:1] * (s * W64[:, 4])[None, :]
         + m[:, 1:2] * (s * W64[:, 5])[None, :]
         + m[:, 2:3] * (s * W64[:, 6])[None, :])
    Q = (q - c0[None, :]).astype(np.float32)                    # [P,64]
    C = np.where((npts < N_PTS)[:, None], c0[None, :], -1e30).astype(np.float32)

    Vmod = voxels.astype(np.float16).copy()
    invalid = np.arange(N_PTS)[None, :] >= npts[:, None]
    Vmod[invalid] = np.broadcast_to(Vmod[:, 0:1, :], Vmod.shape)[invalid]

    pad = P_PAD - P_FULL
    Vp = np.concatenate([Vmod, np.zeros((pad, N_PTS, 4), np.float16)], axis=0)
    Qp = np.concatenate([Q, np.zeros((pad, C_OUT), np.float32)], axis=0)
    Cp = np.concatenate([C, np.zeros((pad, C_OUT), np.float32)], axis=0)
    np_pad = np.concatenate([npts, np.ones(pad, np.int64)])

    order = np.argsort(-np_pad, kind="stable")
    ns = np_pad[order]
    sched = tuple(int(ns[N_CORES * TILE_P * i]) for i in range(N_SLOTS))

    # stationaries: S[32g+4j+c, w, m] = A[c, m%64] if j == 2w + m//64
    A16 = A.astype(np.float16)
    S_small = np.zeros((32, 4, 128), np.float16)
    for w in range(4):
        for half in range(2):
            jj = 2 * w + half
            S_small[4 * jj : 4 * jj + 4, w, 64 * half : 64 * half + 64] = A16
    S = np.tile(S_small, (4, 1, 1))  # [128,4,128]

    Vs = Vp[order]
    in_maps = []
    for k in range(N_CORES):
        mp = {"S": S}
        for i, maxN in enumerate(sched):
            G = (maxN + 7) // 8
            c = N_CORES * i + k
            sl = slice(TILE_P * c, TILE_P * (c + 1))
            mp[f"T{i}"] = np.ascontiguousarray(
                Vs[sl][:, : 8 * G, :].transpose(1, 2, 0).reshape(32 * G, TILE_P)
            )
        in_maps.append(mp)
    return in_maps, sched, order, Qp[order], Cp[order]


def _gather(results, sched, order, Qs, Cs):
    smax = np.empty((P_PAD, C_OUT), np.float32)
    for k in range(N_CORES):
        for i in range(N_SLOTS):
            Ok = results[k][f"O{i}"]  # [128, n_planes, 512] fp16
            pm = Ok.max(axis=1)       # [128, 512]
            fold = np.maximum(pm[:C_OUT, :], pm[C_OUT:, :]).astype(np.float32)
            c = N_CORES * i + k
            smax[TILE_P * c : TILE_P * (c + 1)] = fold.T
    out_sorted = np.maximum(np.maximum(smax - Qs, Cs), 0.0)
    out_full = np.empty_like(out_sorted)
    out_full[order] = out_sorted
    return np.ascontiguousarray(out_full[:P_FULL])


def kernel(**inputs):
    from concourse.bass_utils import run_bass_kernel_spmd

    in_maps, sched, order, Qs, Cs = _plan(**inputs)
    if sched not in _CACHE:
        _CACHE[sched] = _build_nc(sched)
    res = run_bass_kernel_spmd(_CACHE[sched], in_maps, list(range(N_CORES)))
    return _gather(res.results, sched, order, Qs, Cs)


# revision 9
# speedup vs baseline: 2.2048x; 1.1953x over previous
"""PillarVFE on 8 trn2 NeuronCores — v6: fp16 matmuls + Act/DVE chain-pair
PSUM drain, plane outputs, epilogue on host.

Math: per pillar p, point n with raw r=(x,y,z,w):
  out[p,o] = relu( max( max_n (r_n . A)[o] - Q_p[o],  C_p[o] ) )
where A[4,64] folds W + BN scale, Q_p folds the pillar-constant part
(center offsets + cluster mean) minus the BN bias, and C_p is the
candidate from masked points: c0 if npts<32 else -inf.  The device
computes partial maxes of S_p[o] = max_n (r_n . A)[o]; the cheap
elementwise epilogue (plane fold, half fold, -Q, max C, relu,
unpermute) runs on host.

Device structure: pillars sorted by npts desc, 10 slots x 512 pillars
per core; slot i runs B=ceil(maxN_i/2) point-pair matmuls (partition =
2x64 channels, free = 512 pillars) into 2-bank PSUM tiles.  PSUM tiles
drain in PAIRS: Act copy-casts tile A to fp16 SBUF (one 1024-col op),
then DVE does max(cast_A, psum_B) in one mixed-dtype op -> 2 fp16
planes.  4 banks retired per (1112ns Act + 1192ns DVE) running
concurrently on different pairs; PE, Act, DVE all ~saturated.  Lone
tiles drain via Act cast (2 planes) or DVE reduce (1 plane), chosen by
a static load balancer.  Planes collect in a per-slot out tile ->
HBM; host does the remaining small folds.  All T inputs prefetch at
kernel start.  Empirical constraints: TensorTensor reads at most one
PSUM operand; GPSIMD can't touch PSUM or run TensorTensor; no
cross-partition ops; strided Act writes 5x slow; fp16 TT gets 2x mode,
reduce does not.
"""

import sys

import numpy as np

sys.path.insert(0, "/opt/trn_rl_repo")

VX, VY = 0.16, 0.16
X_OFF = VX / 2 + 0.0
Y_OFF = VY / 2 + (-39.68)
BN_EPS = 1e-3

P_FULL = 40000
N_PTS = 32
C_OUT = 64
N_CORES = 8
N_SLOTS = 10
TILE_P = 512
P_PAD = N_CORES * N_SLOTS * TILE_P  # 40960

_CACHE = {}


def _slot_items(maxN):
    """Drain items per slot: 'p' = pair of 2-bank tiles, 'f' = lone
    2-bank tile, 'h' = 1-bank tile."""
    B = (maxN + 1) // 2
    t2, rem = B // 2, B % 2
    items = ["p"] * (t2 // 2)
    if t2 % 2:
        items.append("f")
    if rem:
        items.append("h")
    return items, B


def _plan_slots(sched):
    """Static engine assignment for lone tiles; plane layout per slot."""
    est_v, est_a = 0.0, 0.0
    plan = []
    for maxN in sched:
        items, B = _slot_items(maxN)
        n_planes = 0
        owners = []
        for it in items:
            if it == "p":
                est_a += 1112.0
                est_v += 1192.0
                owners.append("p")
                n_planes += 2
            elif it == "f":
                if est_v + 1223.0 <= est_a + 1112.0:
                    owners.append("fv")       # DVE reduce -> 1 plane
                    est_v += 1223.0
                    n_planes += 1
                else:
                    owners.append("fa")       # Act cast -> 2 planes
                    est_a += 1112.0
                    n_planes += 2
            else:  # 'h'
                if est_v + 690.0 <= est_a + 570.0:
                    owners.append("hv")
                    est_v += 690.0
                else:
                    owners.append("ha")
                    est_a += 570.0
                n_planes += 1
        plan.append((n_planes, owners))
    _plan_slots.est = (est_v, est_a)
    return plan


def _build_nc(sched):
    from contextlib import ExitStack

    from concourse import bass, tile
    from concourse import mybir

    f32 = mybir.dt.float32
    f16 = mybir.dt.float16
    nc = bass.Bass()

    plan = _plan_slots(sched)

    T_ds = []
    for i, maxN in enumerate(sched):
        G = (maxN + 7) // 8
        T_ds.append(
            nc.dram_tensor(f"T{i}", [32 * G, TILE_P], f16, kind="ExternalInput")
        )
    S_d = nc.dram_tensor("S", [128, 4, 128], f16, kind="ExternalInput")
    O_ds = [
        nc.dram_tensor(f"O{i}", [128, n_planes, TILE_P], f16, kind="ExternalOutput")
        for i, (n_planes, _) in enumerate(plan)
    ]

    with tile.TileContext(nc) as tc, ExitStack() as ctx:
        stat = ctx.enter_context(tc.tile_pool(name="stat", bufs=1))
        upool = ctx.enter_context(tc.tile_pool(name="upool", bufs=4))
        opool = ctx.enter_context(tc.tile_pool(name="opool", bufs=2))
        psum = ctx.enter_context(
            tc.tile_pool(name="ps", bufs=4, space=bass.MemorySpace.PSUM)
        )

        # prefetch stationaries + ALL slot inputs up front
        s_sb = stat.tile([128, 4, 128], f16)
        nc.sync.dma_start(s_sb[:], S_d[:])
        t_sbs = []
        for i, maxN in enumerate(sched):
            G = (maxN + 7) // 8
            t_sb = stat.tile([32 * G, TILE_P], f16, name=f"t{i}")
            nc.sync.dma_start(t_sb[:], T_ds[i][:])
            t_sbs.append(t_sb)

        for i, maxN in enumerate(sched):
            G = (maxN + 7) // 8
            n_planes, owners = plan[i]
            t_sb = t_sbs[i]

            pairs = [
                (w, g) for w in range(4) for g in range(G) if 8 * g + 2 * w < maxN
            ]
            B = (maxN + 1) // 2
            assert len(pairs) == B, (i, maxN, pairs)

            def mm2(pt, bank, j):
                w, g = pairs[j]
                nc.tensor.matmul(
                    pt[:, bank, :],
                    s_sb[32 * g : 32 * g + 32, w, :],
                    t_sb[32 * g : 32 * g + 32, :],
                    start=True,
                    stop=True,
                    tile_position=(32 * g, 0),
                )

            out_sb = opool.tile([128, n_planes, TILE_P], f16, name="o")
            idx = 0
            j = 0
            for it in owners:
                if it == "p":
                    pa = psum.tile([128, 2, TILE_P], f32, name="pt")
                    mm2(pa, 0, j); mm2(pa, 1, j + 1)
                    pb = psum.tile([128, 2, TILE_P], f32, name="pt")
                    mm2(pb, 0, j + 2); mm2(pb, 1, j + 3)
                    j += 4
                    u = upool.tile([128, 2, TILE_P], f16, name="u")
                    nc.scalar.copy(u[:], pa[:])
                    nc.vector.tensor_max(
                        out_sb[:, idx : idx + 2, :], u[:], pb[:]
                    )
                    idx += 2
                elif it in ("fv", "fa"):
                    pt = psum.tile([128, 2, TILE_P], f32, name="pt")
                    mm2(pt, 0, j); mm2(pt, 1, j + 1)
                    j += 2
                    if it == "fv":
                        nc.vector.tensor_reduce(
                            out_sb[:, idx, :],
                            pt[:].transpose([0, 2, 1]),
                            axis=mybir.AxisListType.X,
                            op=mybir.AluOpType.max,
                        )
                        idx += 1
                    else:
                        nc.scalar.copy(out_sb[:, idx : idx + 2, :], pt[:])
                        idx += 2
                else:  # 'hv' / 'ha'
                    pt = psum.tile([128, 2, TILE_P], f32, name="pt")
                    mm2(pt, 0, j)
                    j += 1
                    if it == "hv":
                        nc.vector.tensor_copy(out_sb[:, idx, :], pt[:, 0, :])
                    else:
                        nc.scalar.copy(out_sb[:, idx, :], pt[:, 0, :])
                    idx += 1
            assert idx == n_planes and j == B, (i, idx, n_planes, j, B)
            nc.sync.dma_start(O_ds[i][:], out_sb[:])

    nc.finalize()
    import bass_rust

    # walrus codegen allows at most 1 sync wait per instruction
    bass_rust.generate_event_semaphores(nc)
    return nc


def _plan(voxels, W, gamma, beta, running_mean, running_var,
          voxel_num_points, voxel_coords):
    V = voxels.astype(np.float64)
    npts = voxel_num_points.astype(np.int64)
    coords = voxel_coords.astype(np.float64)
    W64 = W.astype(np.float64)
    s = gamma.astype(np.float64) / np.sqrt(running_var.astype(np.float64) + BN_EPS)
    c0 = beta.astype(np.float64) - running_mean.astype(np.float64) * s

    A = np.stack([
        s * (W64[:, 0] + W64[:, 4] + W64[:, 7]),
        s * (W64[:, 1] + W64[:, 5] + W64[:, 8]),
        s * (W64[:, 2] + W64[:, 6]),
        s * W64[:, 3],
    ], axis=0)  # [4,64]

    cx = coords[:, 3] * VX + X_OFF
    cy = coords[:, 2] * VY + Y_OFF
    m = V[:, :, :3].sum(axis=1) / npts[:, None]
    q = (cx[:, None] * (s * (W64[:, 0] + W64[:, 7]))[None, :]
         + cy[:, None] * (s * (W64[:, 1] + W64[:, 8]))[None, :]
         + m[:, 0:1] * (s * W64[:, 4])[None, :]
         + m[:, 1:2] * (s * W64[:, 5])[None, :]
         + m[:, 2:3] * (s * W64[:, 6])[None, :])
    Q = (q - c0[None, :]).astype(np.float32)                    # [P,64]
    C = np.where((npts < N_PTS)[:, None], c0[None, :], -1e30).astype(np.float32)

    Vmod = voxels.astype(np.float16).copy()
    invalid = np.arange(N_PTS)[None, :] >= npts[:, None]
    Vmod[invalid] = np.broadcast_to(Vmod[:, 0:1, :], Vmod.shape)[invalid]

    pad = P_PAD - P_FULL
    Vp = np.concatenate([Vmod, np.zeros((pad, N_PTS, 4), np.float16)], axis=0)
    Qp = np.concatenate([Q, np.zeros((pad, C_OUT), np.float32)], axis=0)
    Cp = np.concatenate([C, np.zeros((pad, C_OUT), np.float32)], axis=0)
    np_pad = np.concatenate([npts, np.ones(pad, np.int64)])

    order = np.argsort(-np_pad, kind="stable")
    ns = np_pad[order]
    sched = tuple(int(ns[N_CORES * TILE_P * i]) for i in range(N_SLOTS))

    # stationaries: S[32g+4j+c, w, m] = A[c, m%64] if j == 2w + m//64
    A16 = A.astype(np.float16)
    S_small = np.zeros((32, 4, 128), np.float16)
    for w in range(4):
        for half in range(2):
            jj = 2 * w + half
            S_small[4 * jj : 4 * jj + 4, w, 64 * half : 64 * half + 64] = A16
    S = np.tile(S_small, (4, 1, 1))  # [128,4,128]

    Vs = Vp[order]
    in_maps = []
    for k in range(N_CORES):
        mp = {"S": S}
        for i, maxN in enumerate(sched):
            G = (maxN + 7) // 8
            c = N_CORES * i + k
            sl = slice(TILE_P * c, TILE_P * (c + 1))
            mp[f"T{i}"] = np.ascontiguousarray(
                Vs[sl][:, : 8 * G, :].transpose(1, 2, 0).reshape(32 * G, TILE_P)
            )
        in_maps.append(mp)
    return in_maps, sched, order, Qp[order], Cp[order]


def _gather(results, sched, order, Qs, Cs):
    smax = np.empty((P_PAD, C_OUT), np.float32)
    for k in range(N_CORES):
        for i in range(N_SLOTS):
            Ok = results[k][f"O{i}"]  # [128, n_planes, 512] fp16
            pm = Ok.max(axis=1)       # [128, 512]
            fold = np.maximum(pm[:C_OUT, :], pm[C_OUT:, :]).astype(np.float32)
            c = N_CORES * i + k
            smax[TILE_P * c : TILE_P * (c + 1)] = fold.T
    out_sorted = np.maximum(np.maximum(smax - Qs, Cs), 0.0)
    out_full = np.empty_like(out_sorted)
    out_full[order] = out_sorted
    return np.ascontiguousarray(out_full[:P_FULL])


def kernel(**inputs):
    from concourse.bass_utils import run_bass_kernel_spmd

    in_maps, sched, order, Qs, Cs = _plan(**inputs)
    if sched not in _CACHE:
        _CACHE[sched] = _build_nc(sched)
    res = run_bass_kernel_spmd(_CACHE[sched], in_maps, list(range(N_CORES)))
    return _gather(res.results, sched, order, Qs, Cs)
